# revision 4
# baseline (speedup 1.0000x reference)
"""MoE linear kernel for Trainium2, 8 NeuronCores, data-parallel over batch.

Problem (hardcoded shapes):
  x  [8192, 1024] f32, Wg [1024, 16], bg [16], We [16, 1024, 1024], be [16, 1024]
  out[b, o] = sum_e softmax(x @ Wg + bg)[b, e] * (x @ We[e] + be[e])[b, o]

Per-core (batch shard of 1024 rows):
  - PE-transpose x shard -> xT (i on partitions) once; reuse as matmul lhsT.
  - Gate logits via small matmuls, softmax on DVE/ACT.
  - Bias term (gate @ be) seeds the output accumulators via one K=16 matmul
    per output tile.
  - Expert loop streams We once; 8 k-chunk matmuls accumulate in PSUM
    ([128 b x 512 o] per group, fp32r at 1 cycle/row); DVE does
    acc = psum * gate[:, e] + acc (single fused scalar_tensor_tensor).
"""

import numpy as np

import concourse.bass as bass
import concourse.mybir as mybir
import concourse.tile as tile
from concourse.bass_utils import run_bass_kernel_spmd
from concourse.masks import make_identity

P = 128
B, D_IN, D_OUT, E = 8192, 1024, 1024, 16
NCORES = 8
BSH = B // NCORES          # 1024 batch rows per core
BT = BSH // P              # 8 batch tiles per core
KC = D_IN // P             # 8 contraction chunks
OH = 2                     # output halves
ON = D_OUT // OH           # 512 output cols per matmul group

F32 = mybir.dt.float32
F32R = mybir.dt.float32r


def _split_multi_waits(nc, limit=1):
    """The walrus build in this container rejects instructions carrying more
    than `limit` semaphore waits ("Too many sync wait commands" on the Tile
    tail drain). Move extra waits onto preceding same-engine NoOps."""
    n = 0
    for f in nc.m.functions:
        for bb in f.blocks:
            insts = bb.instructions
            i = 0
            while i < len(insts):
                ins = insts[i]
                si = ins.sync_info
                if si is not None and len(si.on_wait) > limit:
                    waits = list(si.on_wait)
                    extra, keep = waits[:-limit], waits[-limit:]
                    for j in range(0, len(extra), limit):
                        nop = mybir.InstNoOp(
                            name=f"I-waitsplit-{n}",
                            engine=ins.engine,
                            sync_info=mybir.SyncInfo(
                                on_wait=list(extra[j : j + limit]), on_update=[]
                            ),
                        )
                        n += 1
                        insts.insert(i, nop)
                        i += 1
                    si.on_wait = keep
                i += 1
    return n


def _r(ap):
    """fp32r view of an fp32 access pattern (PE full-rate fp32 matmul)."""
    return ap.bitcast(F32R)


def _build():
    nc = bass.Bass(trn_type="TRN2")

    xs = nc.dram_tensor("xs", [BSH, D_IN], F32, kind="ExternalInput")
    wg = nc.dram_tensor("wg", [D_IN, E], F32, kind="ExternalInput")
    bgb = nc.dram_tensor("bgb", [P, E], F32, kind="ExternalInput")  # bg row-bcast
    we = nc.dram_tensor("we", [E, D_IN, D_OUT], F32, kind="ExternalInput")
    be = nc.dram_tensor("be", [E, D_OUT], F32, kind="ExternalInput")
    out = nc.dram_tensor("out", [BSH, D_OUT], F32, kind="ExternalOutput")

    with tile.TileContext(nc) as tc:
        with (
            tc.tile_pool(name="persist", bufs=1) as persist,
            tc.tile_pool(name="xin", bufs=3) as xin_pool,
            tc.tile_pool(name="wes", bufs=3) as we_pool,
            tc.tile_pool(name="sm", bufs=2) as sm_pool,
            tc.tile_pool(name="psum", bufs=4, space="PSUM") as psum,
            tc.tile_pool(name="tpsum", bufs=2, space="PSUM") as tpsum,
            tc.tile_pool(name="gpsum", bufs=1, space="PSUM") as gpsum,
        ):
            ident = persist.tile([P, P], F32, tag="ident")
            make_identity(nc, ident[:])

            # Replicated small tensors
            wg_s = persist.tile([P, KC, E], F32R, tag="wg")
            nc.sync.dma_start(wg_s[:], wg.rearrange("(kc p) e -> p kc e", p=P).bitcast(F32R))
            bg_s = persist.tile([P, E], F32, tag="bg")
            nc.sync.dma_start(bg_s[:], bgb[:])
            be_s = persist.tile([E, D_OUT], F32R, tag="be")
            nc.sync.dma_start(be_s[:], be[:].bitcast(F32R))

            # Persistent per-core tensors
            xT = [persist.tile([P, KC, P], F32R, tag=f"xT{bt}", name=f"xT{bt}") for bt in range(BT)]
            gate = [persist.tile([P, E], F32, tag=f"g{bt}", name=f"g{bt}") for bt in range(BT)]
            gateT = persist.tile([E, BSH], F32R, tag="gateT")
            acc = [
                [persist.tile([P, ON], F32, tag=f"acc{bt}_{oh}", name=f"acc{bt}_{oh}") for oh in range(OH)]
                for bt in range(BT)
            ]

            # ---- Phase A: transpose x, gate logits + softmax, gate^T ----
            for bt in range(BT):
                xin = xin_pool.tile([P, D_IN], F32, tag="xin")
                nc.sync.dma_start(xin[:], xs[bt * P : (bt + 1) * P, :])
                for kc in range(KC):
                    tp = tpsum.tile([P, P], F32, tag="tp")
                    nc.tensor.transpose(tp[:], xin[:, kc * P : (kc + 1) * P], ident[:])
                    nc.vector.tensor_copy(xT[bt][:, kc, :], tp[:])

                pg = gpsum.tile([P, E], F32, tag="pg")
                for kc in range(KC):
                    nc.tensor.matmul(
                        pg[:],
                        _r(xT[bt][:, kc, :]),
                        _r(wg_s[:, kc, :]),
                        start=(kc == 0),
                        stop=(kc == KC - 1),
                    )
                logits = sm_pool.tile([P, E], F32, tag="logits")
                nc.vector.tensor_add(logits[:], pg[:], bg_s[:])
                negmax = sm_pool.tile([P, 1], F32, tag="negmax")
                nc.vector.tensor_reduce(
                    out=negmax[:],
                    in_=logits[:],
                    op=mybir.AluOpType.max,
                    axis=mybir.AxisListType.X,
                    negate=True,
                )
                esum = sm_pool.tile([P, 1], F32, tag="esum")
                nc.scalar.activation(
                    gate[bt][:],
                    logits[:],
                    mybir.ActivationFunctionType.Exp,
                    bias=negmax[:, 0:1],
                    accum_out=esum[:, 0:1],
                )
                rsum = sm_pool.tile([P, 1], F32, tag="rsum")
                nc.vector.reciprocal(rsum[:], esum[:])
                nc.vector.tensor_scalar_mul(gate[bt][:], gate[bt][:], rsum[:, 0:1])

                gtp = gpsum.tile([E, P], F32, tag="gtp")
                nc.tensor.transpose(gtp[:], gate[bt][:], ident[:])
                nc.vector.tensor_copy(gateT[:, bt * P : (bt + 1) * P], gtp[:])

            # ---- Phase A.5: seed accumulators with gate @ be ----
            for bt in range(BT):
                for oh in range(OH):
                    psb = psum.tile([P, ON], F32, tag="ps")
                    nc.tensor.matmul(
                        psb[:],
                        _r(gateT[:, bt * P : (bt + 1) * P]),
                        _r(be_s[:, oh * ON : (oh + 1) * ON]),
                        start=True,
                        stop=True,
                    )
                    nc.vector.tensor_copy(acc[bt][oh][:], psb[:])

            # ---- Phase B: expert loop (We streamed once) ----
            for e in range(E):
                for oh in range(OH):
                    wt = we_pool.tile([P, KC, ON], F32R, tag="we")
                    nc.sync.dma_start(
                        wt[:],
                        we[e].rearrange("(kc p) o -> p kc o", p=P)[
                            :, :, oh * ON : (oh + 1) * ON
                        ].bitcast(F32R),
                    )
                    for bt in range(BT):
                        ps = psum.tile([P, ON], F32, tag="ps")
                        for kc in range(KC):
                            nc.tensor.matmul(
                                ps[:],
                                _r(xT[bt][:, kc, :]),
                                _r(wt[:, kc, :]),
                                start=(kc == 0),
                                stop=(kc == KC - 1),
                            )
                        nc.vector.scalar_tensor_tensor(
                            out=acc[bt][oh][:],
                            in0=ps[:],
                            scalar=gate[bt][:, e : e + 1],
                            in1=acc[bt][oh][:],
                            op0=mybir.AluOpType.mult,
                            op1=mybir.AluOpType.add,
                        )

            # ---- Phase C: store ----
            for bt in range(BT):
                for oh in range(OH):
                    nc.sync.dma_start(
                        out[bt * P : (bt + 1) * P, oh * ON : (oh + 1) * ON],
                        acc[bt][oh][:],
                    )

    _split_multi_waits(nc)
    return nc


_CACHE = {}


def _get_nc():
    if "nc" not in _CACHE:
        _CACHE["nc"] = _build()
    return _CACHE["nc"]


def make_in_maps(x, Wg, bg, We, be):
    x = np.ascontiguousarray(np.asarray(x, dtype=np.float32))
    Wg = np.ascontiguousarray(np.asarray(Wg, dtype=np.float32))
    bg = np.asarray(bg, dtype=np.float32).reshape(E)
    We = np.ascontiguousarray(np.asarray(We, dtype=np.float32))
    be = np.ascontiguousarray(np.asarray(be, dtype=np.float32))
    bgb = np.ascontiguousarray(np.broadcast_to(bg[None, :], (P, E)))
    return [
        {
            "xs": x[c * BSH : (c + 1) * BSH],
            "wg": Wg,
            "bgb": bgb,
            "we": We,
            "be": be,
        }
        for c in range(NCORES)
    ]


def kernel(x, Wg, bg, We, be):
    nc = _get_nc()
    in_maps = make_in_maps(x, Wg, bg, We, be)
    res = run_bass_kernel_spmd(nc, in_maps, core_ids=list(range(NCORES)))
    return np.concatenate([r["out"] for r in res.results], axis=0)


# revision 5
# speedup vs baseline: 32146.5565x; 32146.5565x over previous
"""MoE linear kernel for Trainium2, 8 NeuronCores, data-parallel over batch.

Problem (hardcoded shapes):
  x  [8192, 1024] f32, Wg [1024, 16], bg [16], We [16, 1024, 1024], be [16, 1024]
  out[b, o] = sum_e softmax(x @ Wg + bg)[b, e] * (x @ We[e] + be[e])[b, o]

Per-core (batch shard of 1024 rows):
  - PE-transpose x shard -> xT (i on partitions) once; reuse as matmul lhsT.
  - Gate logits via small matmuls, softmax on DVE/ACT.
  - Bias term (gate @ be) seeds the output accumulators via one K=16 matmul
    per output tile.
  - Expert loop streams We once; 8 k-chunk matmuls accumulate in PSUM
    ([128 b x 512 o] per group, fp32r at 1 cycle/row); DVE does
    acc = psum * gate[:, e] + acc (single fused scalar_tensor_tensor).
"""

import numpy as np

import concourse.bass as bass
import concourse.mybir as mybir
import concourse.tile as tile
from concourse.bass_utils import run_bass_kernel_spmd
from concourse.masks import make_identity

P = 128
B, D_IN, D_OUT, E = 8192, 1024, 1024, 16
NCORES = 8
BSH = B // NCORES          # 1024 batch rows per core
BT = BSH // P              # 8 batch tiles per core
KC = D_IN // P             # 8 contraction chunks
OH = 2                     # output halves
ON = D_OUT // OH           # 512 output cols per matmul group

F32 = mybir.dt.float32
F32R = mybir.dt.float32r


def _split_multi_waits(nc, limit=1):
    """The walrus build in this container rejects instructions carrying more
    than `limit` semaphore waits ("Too many sync wait commands" on the Tile
    tail drain). Move extra waits onto preceding same-engine NoOps."""
    n = 0
    for f in nc.m.functions:
        for bb in f.blocks:
            insts = bb.instructions
            i = 0
            while i < len(insts):
                ins = insts[i]
                si = ins.sync_info
                if si is not None and len(si.on_wait) > limit:
                    waits = list(si.on_wait)
                    extra, keep = waits[:-limit], waits[-limit:]
                    for j in range(0, len(extra), limit):
                        nop = mybir.InstNoOp(
                            name=f"I-waitsplit-{n}",
                            engine=ins.engine,
                            sync_info=mybir.SyncInfo(
                                on_wait=list(extra[j : j + limit]), on_update=[]
                            ),
                        )
                        n += 1
                        insts.insert(i, nop)
                        i += 1
                    si.on_wait = keep
                i += 1
    return n


def _r(ap):
    """fp32r view of an fp32 access pattern (PE full-rate fp32 matmul)."""
    return ap.bitcast(F32R)


def _build(repeat=1):
    nc = bass.Bass(trn_type="TRN2")

    xs = nc.dram_tensor("xs", [BSH, D_IN], F32, kind="ExternalInput")
    wg = nc.dram_tensor("wg", [D_IN, E], F32, kind="ExternalInput")
    bgb = nc.dram_tensor("bgb", [P, E], F32, kind="ExternalInput")  # bg row-bcast
    we = nc.dram_tensor("we", [E, D_IN, D_OUT], F32, kind="ExternalInput")
    be = nc.dram_tensor("be", [E, D_OUT], F32, kind="ExternalInput")
    out = nc.dram_tensor("out", [BSH, D_OUT], F32, kind="ExternalOutput")

    with tile.TileContext(nc) as tc:
      for _rep in range(repeat):
        with (
            tc.tile_pool(name="persist", bufs=1) as persist,
            tc.tile_pool(name="xin", bufs=3) as xin_pool,
            tc.tile_pool(name="wes", bufs=3) as we_pool,
            tc.tile_pool(name="sm", bufs=2) as sm_pool,
            tc.tile_pool(name="psum", bufs=4, space="PSUM") as psum,
            tc.tile_pool(name="tpsum", bufs=2, space="PSUM") as tpsum,
            tc.tile_pool(name="gpsum", bufs=1, space="PSUM") as gpsum,
        ):
            ident = persist.tile([P, P], F32, tag="ident")
            make_identity(nc, ident[:])

            # Replicated small tensors
            wg_s = persist.tile([P, KC, E], F32R, tag="wg")
            nc.sync.dma_start(wg_s[:], wg.rearrange("(kc p) e -> p kc e", p=P).bitcast(F32R))
            bg_s = persist.tile([P, E], F32, tag="bg")
            nc.sync.dma_start(bg_s[:], bgb[:])
            be_s = persist.tile([E, D_OUT], F32R, tag="be")
            nc.sync.dma_start(be_s[:], be[:].bitcast(F32R))

            # Persistent per-core tensors
            xT = [persist.tile([P, KC, P], F32R, tag=f"xT{bt}", name=f"xT{bt}") for bt in range(BT)]
            gate = [persist.tile([P, E], F32, tag=f"g{bt}", name=f"g{bt}") for bt in range(BT)]
            gateT = persist.tile([E, BSH], F32R, tag="gateT")
            acc = [
                [persist.tile([P, ON], F32, tag=f"acc{bt}_{oh}", name=f"acc{bt}_{oh}") for oh in range(OH)]
                for bt in range(BT)
            ]

            # ---- Phase A: transpose x, gate logits + softmax, gate^T ----
            for bt in range(BT):
                xin = xin_pool.tile([P, D_IN], F32, tag="xin")
                nc.sync.dma_start(xin[:], xs[bt * P : (bt + 1) * P, :])
                for kc in range(KC):
                    tp = tpsum.tile([P, P], F32, tag="tp")
                    nc.tensor.transpose(tp[:], xin[:, kc * P : (kc + 1) * P], ident[:])
                    nc.vector.tensor_copy(xT[bt][:, kc, :], tp[:])

                pg = gpsum.tile([P, E], F32, tag="pg")
                for kc in range(KC):
                    nc.tensor.matmul(
                        pg[:],
                        _r(xT[bt][:, kc, :]),
                        _r(wg_s[:, kc, :]),
                        start=(kc == 0),
                        stop=(kc == KC - 1),
                    )
                logits = sm_pool.tile([P, E], F32, tag="logits")
                nc.vector.tensor_add(logits[:], pg[:], bg_s[:])
                negmax = sm_pool.tile([P, 1], F32, tag="negmax")
                nc.vector.tensor_reduce(
                    out=negmax[:],
                    in_=logits[:],
                    op=mybir.AluOpType.max,
                    axis=mybir.AxisListType.X,
                    negate=True,
                )
                esum = sm_pool.tile([P, 1], F32, tag="esum")
                nc.scalar.activation(
                    gate[bt][:],
                    logits[:],
                    mybir.ActivationFunctionType.Exp,
                    bias=negmax[:, 0:1],
                    accum_out=esum[:, 0:1],
                )
                rsum = sm_pool.tile([P, 1], F32, tag="rsum")
                nc.vector.reciprocal(rsum[:], esum[:])
                nc.vector.tensor_scalar_mul(gate[bt][:], gate[bt][:], rsum[:, 0:1])

                gtp = gpsum.tile([E, P], F32, tag="gtp")
                nc.tensor.transpose(gtp[:], gate[bt][:], ident[:])
                nc.vector.tensor_copy(gateT[:, bt * P : (bt + 1) * P], gtp[:])

            # ---- Phase A.5: seed accumulators with gate @ be ----
            for bt in range(BT):
                for oh in range(OH):
                    psb = psum.tile([P, ON], F32, tag="ps")
                    nc.tensor.matmul(
                        psb[:],
                        _r(gateT[:, bt * P : (bt + 1) * P]),
                        _r(be_s[:, oh * ON : (oh + 1) * ON]),
                        start=True,
                        stop=True,
                    )
                    nc.vector.tensor_copy(acc[bt][oh][:], psb[:])

            # ---- Phase B: expert loop (We streamed once) ----
            for e in range(E):
                for oh in range(OH):
                    wt = we_pool.tile([P, KC, ON], F32R, tag="we")
                    nc.sync.dma_start(
                        wt[:],
                        we[e].rearrange("(kc p) o -> p kc o", p=P)[
                            :, :, oh * ON : (oh + 1) * ON
                        ].bitcast(F32R),
                    )
                    for bt in range(BT):
                        ps = psum.tile([P, ON], F32, tag="ps")
                        for kc in range(KC):
                            nc.tensor.matmul(
                                ps[:],
                                _r(xT[bt][:, kc, :]),
                                _r(wt[:, kc, :]),
                                start=(kc == 0),
                                stop=(kc == KC - 1),
                            )
                        nc.vector.scalar_tensor_tensor(
                            out=acc[bt][oh][:],
                            in0=ps[:],
                            scalar=gate[bt][:, e : e + 1],
                            in1=acc[bt][oh][:],
                            op0=mybir.AluOpType.mult,
                            op1=mybir.AluOpType.add,
                        )

            # ---- Phase C: store ----
            for bt in range(BT):
                for oh in range(OH):
                    nc.sync.dma_start(
                        out[bt * P : (bt + 1) * P, oh * ON : (oh + 1) * ON],
                        acc[bt][oh][:],
                    )

    _split_multi_waits(nc)
    return nc


_CACHE = {}


def _get_nc(repeat=1):
    key = ("nc", repeat)
    if key not in _CACHE:
        _CACHE[key] = _build(repeat)
    return _CACHE[key]


def make_in_maps(x, Wg, bg, We, be):
    x = np.ascontiguousarray(np.asarray(x, dtype=np.float32))
    Wg = np.ascontiguousarray(np.asarray(Wg, dtype=np.float32))
    bg = np.asarray(bg, dtype=np.float32).reshape(E)
    We = np.ascontiguousarray(np.asarray(We, dtype=np.float32))
    be = np.ascontiguousarray(np.asarray(be, dtype=np.float32))
    bgb = np.ascontiguousarray(np.broadcast_to(bg[None, :], (P, E)))
    return [
        {
            "xs": x[c * BSH : (c + 1) * BSH],
            "wg": Wg,
            "bgb": bgb,
            "we": We,
            "be": be,
        }
        for c in range(NCORES)
    ]


def kernel(x, Wg, bg, We, be):
    nc = _get_nc()
    in_maps = make_in_maps(x, Wg, bg, We, be)
    res = run_bass_kernel_spmd(nc, in_maps, core_ids=list(range(NCORES)))
    return np.concatenate([r["out"] for r in res.results], axis=0)
